# revision 40
# baseline (speedup 1.0000x reference)
"""Bidirectional Mamba block (BiT_MamSleep) on 8 TRN2 NeuronCores.

Sharding: core c handles (batch b = c//2, direction dir = c%2). Each core runs
the full pre-projection + its direction's selective scan in feature-major
layout (features on partitions, time on the free dim); the two cores of a pair
exchange their direction outputs with a pairwise AllReduce (the backward
core time-flips + masks before the exchange), then both compute the tail
(gate multiply, output projection, final LN) redundantly.

Selective scan, d-major layout: partitions = 128 d-channels of one half of
d_inner, one scan per state s (16 states x 2 halves fused on the free axis:
[128, 4096] = half0 | half1, with the recurrence reset at the half boundary
by zeroing the dA column there). dt/dt*u are read in place (no replication);
only the per-state B/C rows are broadcast across partitions, via a small bf16
DRAM bounce. exp(A*dt) runs on ScalarE with the per-partition A column as the
activation scale; the dBu and C multiplies are bf16 tensor_tensor ops on
VectorE (GpSimd shares VectorE's second SBUF port via an exclusive lock, so
offloading there is counterproductive); the 16-state contraction accumulates
with identity-weight bf16 matmuls on TensorE. The causal depthwise conv is
folded into the u-projection as 4 tap-scaled shifted matmuls. All projection
matmuls are bf16 with f32 PSUM accumulation.
"""
import sys

if '/opt/trn_rl_repo' not in sys.path:
    sys.path.insert(0, '/opt/trn_rl_repo')

import ml_dtypes
import numpy as np

import concourse.bass as bass
import concourse.bacc as bacc
import concourse.tile as tile
from concourse import mybir
from concourse.bass_utils import run_bass_kernel_spmd

HID = 128
BATCH = 4
SEQ = 2048
D_STATE = 16
D_CONV = 4
D_INNER = 256
DT_RANK = 8

L = SEQ
C = HID
CW = 512           # matmul / PSUM chunk width
NCH = L // CW
NS = 16            # states; one fused [128, 2*L] scan per state
f32 = mybir.dt.float32
bf16 = mybir.dt.bfloat16
mult = mybir.AluOpType.mult
add = mybir.AluOpType.add
sub = mybir.AluOpType.subtract
AF = mybir.ActivationFunctionType

_PROGRAM = None


def _declare(nc):
    def dp(name, shape, dt=f32):
        return nc.declare_dram_parameter(name, list(shape), dt, isOutput=False)
    p = {}
    p['x'] = dp('x', (C, L))
    for n in ('wlgT', 'wcmT', 'loT'):
        p[n] = dp(n, (C, C), bf16)
    # conv folded into the u-projection: 4 tap-scaled copies of in_w's u-half
    p['inwuT'] = dp('inwuT', (C, D_CONV * 2 * 128), bf16)
    p['inwzT'] = dp('inwzT', (C, 2 * 128), bf16)
    p['xpwT0'] = dp('xpwT0', (128, 80), bf16)   # dtr @0:8, B @32:48, C @64:80
    p['xpwT1'] = dp('xpwT1', (128, 80), bf16)
    p['dtwT'] = dp('dtwT', (DT_RANK, D_INNER), bf16)
    p['outwT0'] = dp('outwT0', (128, C), bf16)
    p['outwT1'] = dp('outwT1', (128, C), bf16)
    p['acols'] = dp('acols', (128, 2 * NS))     # col 2s+h = A[128h:128(h+1), s]
    p['iden'] = dp('iden', (128, 128), bf16)
    for n in ('conv_b', 'dt_b', 'dp_v'):
        p[n] = dp(n, (128, 2))                  # halves in columns
    for n in ('bias_lg', 'bias_cm', 'lo_b', 'ln_g', 'ln_b', 'm_fwd', 'm_bwd'):
        p[n] = dp(n, (C, 1))
    p['y'] = nc.declare_dram_parameter('y', [C, L], f32, isOutput=True)
    return p


class B:
    """Builder state shared by the stage helpers."""


def _proj(b, ps_pool, lhsT, rhs, out, func, bias, out_cols=None, rows=C):
    """out[:, cs] = func(lhsT.T @ rhs[:, cs] + bias) per CW-chunk (PE + ACT)."""
    nc = b.nc
    for ci in range(NCH):
        cs = slice(ci * CW, (ci + 1) * CW)
        ocs = cs if out_cols is None else slice(out_cols + ci * CW, out_cols + (ci + 1) * CW)
        ps = ps_pool.tile([rows, CW], f32, name='bank', tag='bank')
        nc.tensor.matmul(ps, lhsT, rhs[:, cs], start=True, stop=True)
        nc.scalar.activation(out[:, ocs], ps, func, bias=bias)


def _layernorm(b, ps_pool, pool, x_sb, out, pref, width=L):
    """LayerNorm over the 128 channels per column into `out` (any dtype):
    (x - mean) * rsqrt(var + eps). Stats via bf16 ones-matmuls; the mean/rstd
    rows are broadcast back across partitions with K=1 ones-row matmuls.
    Stage-major emission so the in-order engines pipeline across chunks and
    the Ln/Exp activation-table loads happen once, not per chunk."""
    nc = b.nc
    nch = width // CW
    xb = pool.tile([C, width], bf16, name=f'lnxb{pref}', tag=f'lnxb{pref}')
    ex = pool.tile([1, width], bf16, name=f'lnex{pref}', tag=f'lnex{pref}')
    rr_ = pool.tile([1, width], f32, name=f'lnrr{pref}', tag=f'lnrr{pref}')
    nrm0 = pool.tile([C, width], f32, name=f'nrm0{pref}', tag=f'nrm0{pref}')
    sq2 = pool.tile([C, width], bf16, name=f'sq2{pref}', tag=f'sq2{pref}')
    cslices = [slice(ci * CW, (ci + 1) * CW) for ci in range(nch)]
    for cs in cslices:
        nc.vector.tensor_copy(xb[:, cs], x_sb[:, cs])
    ps0s = [ps_pool.tile([1, CW], f32, name='bank', tag='bank') for _ in cslices]
    for cs, ps0 in zip(cslices, ps0s):
        nc.tensor.matmul(ps0, b.ones_col, xb[:, cs], start=True, stop=True)
    for cs, ps0 in zip(cslices, ps0s):
        nc.scalar.activation(ex[:, cs], ps0, AF.Identity, bias=0.0, scale=1.0 / C)
    for ci, cs in enumerate(cslices):
        psb = ps_pool.tile([128, CW], f32, name='bank', tag='bank')
        nc.tensor.matmul(psb, b.ones_row, ex[:, cs], start=True, stop=True)
        nc.vector.scalar_tensor_tensor(nrm0[:, cs], x_sb[:, cs], 1.0, psb, mult, sub)
        nc.vector.tensor_tensor(sq2[:, cs], nrm0[:, cs], nrm0[:, cs], mult)
    psvs = [ps_pool.tile([1, CW], f32, name='bank', tag='bank') for _ in cslices]
    for cs, psv in zip(cslices, psvs):
        nc.tensor.matmul(psv, b.ones_col, sq2[:, cs], start=True, stop=True)
    for cs, psv in zip(cslices, psvs):
        nc.scalar.activation(rr_[:, cs], psv, AF.Ln, bias=b.eps_t[:, :], scale=1.0 / C)
    for cs in cslices:
        nc.scalar.activation(rr_[:, cs], rr_[:, cs], AF.Exp, bias=0.0, scale=-0.5)
    for cs in cslices:
        psr = ps_pool.tile([128, CW], f32, name='bank', tag='bank')
        nc.tensor.matmul(psr, b.ones_row_f, rr_[:, cs], start=True, stop=True)
        nc.vector.scalar_tensor_tensor(out[:, cs], nrm0[:, cs], 1.0, psr, mult, mult)


def _build_body(nc, tc, p, ctx):
    b = B()
    b.nc = nc
    io = ctx.enter_context(tc.tile_pool(name='io', bufs=1))
    b.dram = ctx.enter_context(tc.tile_pool(name='drm', bufs=1, space='DRAM'))

    # x first: its DMA leads the dispatch queue so LN1 starts immediately
    x = io.tile([C, L], f32, name='x', tag='x')
    nc.sync.dma_start(out=x, in_=p['x'][:, :])

    # ---- load weights/vectors (persistent) ----
    W = {}
    for n, shape, dt in (('wlgT', (C, C), bf16),
                         ('wcmT', (C, C), bf16), ('loT', (C, C), bf16),
                         ('inwuT', (C, D_CONV * 2 * 128), bf16),
                         ('inwzT', (C, 2 * 128), bf16),
                         ('xpwT0', (128, 80), bf16), ('xpwT1', (128, 80), bf16),
                         ('dtwT', (8, 256), bf16),
                         ('outwT0', (128, C), bf16), ('outwT1', (128, C), bf16),
                         ('acols', (128, 2 * NS), f32), ('iden', (128, 128), bf16)):
        W[n] = io.tile(list(shape), dt, name=n, tag=n)
        nc.sync.dma_start(out=W[n], in_=p[n][:, :])
    V = {}
    for n in ('conv_b', 'dt_b', 'dp_v'):
        V[n] = io.tile([128, 2], f32, name=n, tag=n)
        nc.sync.dma_start(out=V[n], in_=p[n][:, :])
    for n in ('bias_lg', 'bias_cm', 'lo_b', 'ln_g', 'ln_b', 'm_fwd', 'm_bwd'):
        V[n] = io.tile([C, 1], f32, name=n, tag=n)
        nc.sync.dma_start(out=V[n], in_=p[n][:, :])
    ones_col = io.tile([C, 1], bf16, name='ones_col', tag='ones_col')
    nc.vector.memset(ones_col, 1.0)
    b.ones_col = ones_col
    eps_t = io.tile([1, 1], f32, name='lneps', tag='lneps')
    nc.vector.memset(eps_t, 1e-5)
    b.eps_t = eps_t
    ones_row = io.tile([1, 128], bf16, name='ones_row', tag='ones_row')
    nc.vector.memset(ones_row, 1.0)
    b.ones_row = ones_row
    ones_row_f = io.tile([1, 128], f32, name='ones_row_f', tag='ones_row_f')
    nc.vector.memset(ones_row_f, 1.0)
    b.ones_row_f = ones_row_f

    # persistent activations that survive into the s-loop / tail
    gate = io.tile([C, L], bf16, name='gate', tag='gate')
    b.nrm = io.tile([C, L], bf16, name='nrm', tag='nrm')
    uc = [io.tile([128, L], bf16, name=f'uc{h}', tag=f'uc{h}') for h in range(2)]
    sz_t = [io.tile([128, L], bf16, name=f'sz{h}', tag=f'sz{h}') for h in range(2)]
    dtt = [io.tile([128, L], bf16, name=f'dtt{h}', tag=f'dtt{h}') for h in range(2)]
    dtut = [io.tile([128, L], bf16, name=f'dtut{h}', tag=f'dtut{h}') for h in range(2)]

    b_d = b.dram.tile([NS, L], bf16, name='b_d', tag='b_d')
    c_d = b.dram.tile([NS, L], bf16, name='c_d', tag='c_d')

    # ================= P1/P2: layernorm, projections, conv, dbl =============
    with tc.tile_pool(name='head', bufs=1) as head, \
         tc.tile_pool(name='hps', bufs=4, space='PSUM') as hps:
        _layernorm(b, hps, head, x, b.nrm, 'l1')

        # lm-projection folded into wc on the host (both are linear):
        # xm = silu((wc @ wlm') @ nrm + (wc @ b_lm + cb)),
        # left-padded with D_CONV-1 zero columns for the folded conv
        xmp = head.tile([C, D_CONV - 1 + L], bf16, name='xmp', tag='xmp')
        nc.vector.memset(xmp[:, 0:D_CONV - 1], 0.0)
        _proj(b, hps, W['wcmT'], b.nrm, xmp, AF.Silu, V['bias_cm'][:, :],
              out_cols=D_CONV - 1)

        # z-projection + silu, and the u-projection with the causal depthwise
        # conv folded in: uc[:, t] = silu(sum_k (cw_k*in_w_u) @ xm[:, t-3+k] + cb)
        for h in range(2):
            _proj(b, hps, W['inwzT'][:, 128 * h:128 * (h + 1)], xmp[:, 3:3 + L],
                  sz_t[h], AF.Silu, 0.0)
            for ci in range(NCH):
                cs = slice(ci * CW, (ci + 1) * CW)
                ps_u = hps.tile([128, CW], f32, name='bank', tag='bank')
                for kk in range(D_CONV):
                    wk = W['inwuT'][:, 128 * (4 * h + kk):128 * (4 * h + kk + 1)]
                    nc.tensor.matmul(ps_u, wk, xmp[:, ci * CW + kk:ci * CW + kk + CW],
                                     start=(kk == 0), stop=(kk == D_CONV - 1))
                nc.scalar.activation(uc[h][:, cs], ps_u, AF.Silu,
                                     bias=V['conv_b'][:, h:h + 1])

        # dbl = xp_w @ uc -> dtr(8, bf16), B(16, bf16), Cm(16, bf16)
        dtr = head.tile([8, L], bf16, name='dtr', tag='dtr')
        b_sb = head.tile([16, L], bf16, name='b_sb', tag='b_sb')
        c_sb = head.tile([16, L], bf16, name='c_sb', tag='c_sb')
        for ci in range(NCH):
            cs = slice(ci * CW, (ci + 1) * CW)
            ps_dbl = hps.tile([80, CW], f32, name='bank', tag='bank')
            nc.tensor.matmul(ps_dbl, W['xpwT0'], uc[0][:, cs], start=True, stop=False)
            nc.tensor.matmul(ps_dbl, W['xpwT1'], uc[1][:, cs], start=False, stop=True)
            nc.vector.tensor_copy(dtr[:, cs], ps_dbl[0:8, :])
            nc.vector.tensor_copy(b_sb[:, cs], ps_dbl[32:48, :])
            nc.vector.tensor_copy(c_sb[:, cs], ps_dbl[64:80, :])
            # stash B/C chunks to DRAM for the per-state partition broadcast
            nc.sync.dma_start(out=b_d[:, cs], in_=b_sb[:, cs])
            nc.sync.dma_start(out=c_d[:, cs], in_=c_sb[:, cs])

        # dt = softplus(dt_w @ dtr + dt_b) (bf16); dtu = dt * uc
        # softplus(z) = ln(1 + exp(z)) -- no softplus entry in the ACT tables.
        # Stage-major so the Exp/Ln table loads happen once each.
        for h in range(2):
            for ci in range(NCH):
                cs = slice(ci * CW, (ci + 1) * CW)
                ps_dt = hps.tile([128, CW], f32, name='bank', tag='bank')
                nc.tensor.matmul(ps_dt, W['dtwT'][:, 128 * h:128 * (h + 1)],
                                 dtr[:, cs], start=True, stop=True)
                nc.scalar.activation(dtt[h][:, cs], ps_dt, AF.Exp,
                                     bias=V['dt_b'][:, h:h + 1])
        for h in range(2):
            nc.scalar.activation(dtt[h], dtt[h], AF.Ln, bias=1.0, scale=1.0)
            nc.vector.tensor_tensor(dtut[h], dtt[h], uc[h], mult)

    # ================= P3: selective scan, one fused tile per state =========
    yz = []
    with tc.tile_pool(name='py', bufs=1, space='PSUM') as py, \
         tc.tile_pool(name='rot', bufs=2) as rot:
        psy = [py.tile([128, L], f32, name=f'psy{h}', tag=f'psy{h}') for h in range(2)]
        for s in range(NS):
            b_bc = rot.tile([128, L], bf16, name='b_bc', tag='b_bc')
            src = bass.AP(tensor=b_d.tensor, offset=b_d.offset + s * L,
                          ap=[[0, 128], [1, L]])
            nc.sync.dma_start(out=b_bc, in_=src)
            c_bc = rot.tile([128, L], bf16, name='c_bc', tag='c_bc')
            src = bass.AP(tensor=c_d.tensor, offset=c_d.offset + s * L,
                          ap=[[0, 128], [1, L]])
            nc.gpsimd.dma_start(out=c_bc, in_=src)

            da = rot.tile([128, 2 * L], f32, name='da', tag='da')
            nc.scalar.activation(da[:, 0:L], dtt[0], AF.Exp, bias=0.0,
                                 scale=W['acols'][:, 2 * s:2 * s + 1])
            nc.scalar.activation(da[:, L + 1:2 * L], dtt[1][:, 1:L], AF.Exp,
                                 bias=0.0, scale=W['acols'][:, 2 * s + 1:2 * s + 2])
            # state reset at the half boundary: h_first = 0*h_prev + dBu_first
            nc.vector.memset(da[:, L:L + 1], 0.0)

            dbu = rot.tile([128, 2 * L], bf16, name='dbu', tag='dbu')
            nc.vector.tensor_tensor(dbu[:, 0:L], dtut[0], b_bc, mult)
            nc.vector.tensor_tensor(dbu[:, L:2 * L], dtut[1], b_bc, mult)

            ht = rot.tile([128, 2 * L], bf16, name='ht', tag='ht')
            nc.vector.tensor_tensor_scan(ht, da, dbu, 0.0, mult, add)

            ycm = rot.tile([128, 2 * L], bf16, name='ycm', tag='ycm')
            nc.vector.tensor_tensor(ycm[:, 0:L], ht[:, 0:L], c_bc, mult)
            nc.vector.tensor_tensor(ycm[:, L:2 * L], ht[:, L:2 * L], c_bc, mult)

            for h in range(2):
                for ci in range(NCH):
                    ics = slice(h * L + ci * CW, h * L + (ci + 1) * CW)
                    ocs = slice(ci * CW, (ci + 1) * CW)
                    nc.tensor.matmul(psy[h][:, ocs], W['iden'], ycm[:, ics],
                                     start=(s == 0), stop=(s == NS - 1),
                                     skip_group_check=True)

        # y1 = uc*Dp + psy ; yz = y1 * silu(z)
        for h in range(2):
            yzt = io.tile([128, L], bf16, name=f'yz{h}', tag=f'yz{h}')
            nc.vector.scalar_tensor_tensor(
                yzt, uc[h], V['dp_v'][:, h:h + 1], psy[h], mult, add)
            nc.vector.tensor_tensor(yzt, yzt, sz_t[h], mult)
            yz.append(yzt)

    # ================= P4: out-proj, flip, select, pairwise exchange ========
    with tc.tile_pool(name='tail', bufs=1) as tail, \
         tc.tile_pool(name='tps', bufs=4, space='PSUM') as tps:
        y_dir = tail.tile([C, L], bf16, name='y_dir', tag='y_dir')
        for ci in range(NCH):
            cs = slice(ci * CW, (ci + 1) * CW)
            ps_o = tps.tile([C, CW], f32, name='bank', tag='bank')
            nc.tensor.matmul(ps_o, W['outwT0'], yz[0][:, cs], start=True, stop=False)
            nc.tensor.matmul(ps_o, W['outwT1'], yz[1][:, cs], start=False, stop=True)
            nc.scalar.activation(y_dir[:, cs], ps_o, AF.Identity, bias=0.0)

        y_flip = tail.tile([C, L], bf16, name='y_flip', tag='y_flip')
        nc.vector.tensor_copy(y_flip, y_dir[:, ::-1])
        y_sel = tail.tile([C, L], bf16, name='y_sel', tag='y_sel')
        nc.vector.tensor_scalar_mul(y_sel, y_dir, V['m_fwd'][:, :])
        nc.vector.scalar_tensor_tensor(y_sel, y_flip, V['m_bwd'][:, :], y_sel, mult, add)

        # pairwise ReduceScatter over column halves: even cores get summed
        # cols 0:L/2, odd cores cols L/2:L; the host stitches the halves.
        LH = L // 2
        cc_in = b.dram.tile([2 * C, LH], bf16, name='cc_in', tag='cc_in')
        cc_out = b.dram.tile([C, LH], bf16, name='cc_out', tag='cc_out')
        nc.sync.dma_start(out=cc_in[0:C, :], in_=y_sel[:, 0:LH])
        nc.sync.dma_start(out=cc_in[C:2 * C, :], in_=y_sel[:, LH:L])
        nc.gpsimd.collective_compute(
            'ReduceScatter', add,
            replica_groups=[[0, 1], [2, 3], [4, 5], [6, 7]],
            ins=[cc_in.opt()], outs=[cc_out.opt()])
        # gate projection scheduled here so PE/ACT run it in the CC's shadow
        _proj(b, tps, W['wlgT'], b.nrm, gate, AF.Silu, V['bias_lg'][:, :])
        # core-parity column half of the gate, via the fwd/bwd masks
        ghalf = tail.tile([C, LH], bf16, name='ghalf', tag='ghalf')
        nc.vector.tensor_scalar_mul(ghalf, gate[:, 0:LH], V['m_fwd'][:, :])
        nc.vector.scalar_tensor_tensor(ghalf, gate[:, LH:L], V['m_bwd'][:, :],
                                       ghalf, mult, add)
        y_sum = tail.tile([C, LH], bf16, name='y_sum', tag='y_sum')
        nc.sync.dma_start(out=y_sum, in_=cc_out)

        # ---- P5 tail (half-width): gate multiply, lo projection, final LN --
        g1 = tail.tile([C, LH], bf16, name='g1', tag='g1')
        nc.vector.tensor_tensor(g1, y_sum, ghalf, mult)
        t2 = tail.tile([C, LH], f32, name='t2', tag='t2')
        for ci in range(LH // CW):
            cs = slice(ci * CW, (ci + 1) * CW)
            ps_l = tps.tile([C, CW], f32, name='bank', tag='bank')
            nc.tensor.matmul(ps_l, W['loT'], g1[:, cs], start=True, stop=True)
            nc.scalar.activation(t2[:, cs], ps_l, AF.Identity, bias=V['lo_b'][:, :])

        o1 = tail.tile([C, LH], f32, name='o1', tag='o1')
        _layernorm(b, tps, tail, t2, o1, 'l2', width=LH)
        out_sb = tail.tile([C, LH], f32, name='out_sb', tag='out_sb')
        nc.scalar.activation(out_sb, o1, AF.Identity, bias=V['ln_b'][:, :],
                             scale=V['ln_g'][:, :])
        nc.sync.dma_start(out=p['y'][:, 0:LH], in_=out_sb)


def _build_program():
    import contextlib
    nc = bacc.Bacc('TRN2', target_bir_lowering=False, debug=False, num_devices=8)
    p = _declare(nc)
    with tile.TileContext(nc) as tc:
        with contextlib.ExitStack() as ctx:
            _build_body(nc, tc, p, ctx)
    nc.compile()
    return nc


def _prep_core_inputs(inputs, bidx, d):
    g = lambda n: np.asarray(inputs[n], dtype=np.float32)
    x = g('x')
    ln_g = g('ln_g')
    ln_b = g('ln_b')
    pre = 'mf_' if d == 0 else 'mb_'
    P = lambda n: np.asarray(inputs[pre + n], dtype=np.float32)

    lm_w, lm_b = g('lm_w'), g('lm_b')
    lg_w, lg_b = g('lg_w'), g('lg_b')
    lo_w, lo_b = g('lo_w'), g('lo_b')
    if d == 0:
        wc, cb = g('cf_w'), g('cf_b')
    else:
        wc, cb = np.ascontiguousarray(g('cb_w')[:, ::-1]), g('cb_b')

    A = -np.exp(P('Alog'))                       # (256,16)
    acols = np.zeros((128, 2 * NS), np.float32)  # col 2s+h = A[128h:128(h+1), s]
    for s in range(NS):
        acols[:, 2 * s] = A[0:128, s]
        acols[:, 2 * s + 1] = A[128:256, s]

    halves = lambda v: np.ascontiguousarray(
        np.stack([v[:128], v[128:]], axis=1).astype(np.float32))
    col = lambda v: np.ascontiguousarray(v.astype(np.float32).reshape(-1, 1))
    bf = lambda w: np.ascontiguousarray(w).astype(ml_dtypes.bfloat16)
    T = lambda w: np.ascontiguousarray(w.T.astype(np.float32))

    xpwT40 = np.ascontiguousarray(P('xp_w').T)   # (256,40)
    xpwT = np.zeros((256, 80), np.float32)       # 32-aligned sections for PSUM reads
    xpwT[:, 0:8] = xpwT40[:, 0:8]
    xpwT[:, 32:48] = xpwT40[:, 8:24]
    xpwT[:, 64:80] = xpwT40[:, 24:40]
    outwT = np.ascontiguousarray(P('out_w').T)   # (256,128)
    cwn = P('conv_w')                            # (256,4)
    inw = P('in_w')                              # (512,128): u rows 0:256, z 256:512
    # conv folded into the u-projection: per half h and tap k, (cw_k * in_w_u).T
    inwuT = np.concatenate(
        [np.ascontiguousarray((cwn[128 * h:128 * (h + 1), kk:kk + 1]
                               * inw[128 * h:128 * (h + 1)]).T)
         for h in range(2) for kk in range(D_CONV)], axis=1)
    inwzT = np.ascontiguousarray(inw[256:512].T)

    return {
        'x': np.ascontiguousarray(x[bidx]),
        'wlgT': bf(T(lg_w * ln_g[None, :])),
        'wcmT': bf(T(wc @ (lm_w * ln_g[None, :]))),
        'loT': bf(T(lo_w)),
        'inwuT': bf(inwuT),
        'inwzT': bf(inwzT),
        'xpwT0': bf(xpwT[:128]),
        'xpwT1': bf(xpwT[128:]),
        'dtwT': bf(np.ascontiguousarray(P('dt_w').T)),
        'outwT0': bf(outwT[:128]),
        'outwT1': bf(outwT[128:]),
        'acols': acols,
        'iden': bf(np.eye(128, dtype=np.float32)),
        'conv_b': halves(P('conv_b')),
        'dt_b': halves(P('dt_b')),
        'dp_v': halves(P('D')),
        'bias_lg': col(lg_w @ ln_b + lg_b),
        'bias_cm': col(wc @ (lm_w @ ln_b + lm_b) + cb),
        'lo_b': col(lo_b),
        'ln_g': col(ln_g),
        'ln_b': col(ln_b),
        'm_fwd': np.full((C, 1), 1.0 if d == 0 else 0.0, np.float32),
        'm_bwd': np.full((C, 1), 0.0 if d == 0 else 1.0, np.float32),
    }


def get_program():
    global _PROGRAM
    if _PROGRAM is None:
        _PROGRAM = _build_program()
    return _PROGRAM


def run(inputs, **run_kwargs):
    nc = get_program()
    in_maps = [_prep_core_inputs(inputs, c // 2, c % 2) for c in range(8)]
    res = run_bass_kernel_spmd(nc, in_maps, core_ids=list(range(8)), **run_kwargs)
    # each pair's even core computes output cols 0:L/2, the odd core L/2:L
    out = np.stack(
        [np.concatenate([res.results[2 * b]['y'][:, :L // 2],
                         res.results[2 * b + 1]['y'][:, :L // 2]], axis=1)
         for b in range(BATCH)], axis=0)
    return out, res


def kernel(**inputs) -> np.ndarray:
    out, _ = run(inputs)
    return out.astype(np.float32)


# revision 41
# speedup vs baseline: 1.0231x; 1.0231x over previous
"""Bidirectional Mamba block (BiT_MamSleep) on 8 TRN2 NeuronCores.

Sharding: core c handles (batch b = c//2, direction dir = c%2). Each core runs
the full pre-projection + its direction's selective scan in feature-major
layout (features on partitions, time on the free dim); the two cores of a pair
exchange their direction outputs with a pairwise AllReduce (the backward
core time-flips + masks before the exchange), then both compute the tail
(gate multiply, output projection, final LN) redundantly.

Selective scan, d-major layout: partitions = 128 d-channels of one half of
d_inner, one scan per state s (16 states x 2 halves fused on the free axis:
[128, 4096] = half0 | half1, with the recurrence reset at the half boundary
by zeroing the dA column there). dt/dt*u are read in place (no replication);
only the per-state B/C rows are broadcast across partitions, via a small bf16
DRAM bounce. exp(A*dt) runs on ScalarE with the per-partition A column as the
activation scale; the dBu and C multiplies are bf16 tensor_tensor ops on
VectorE (GpSimd shares VectorE's second SBUF port via an exclusive lock, so
offloading there is counterproductive); the 16-state contraction accumulates
with identity-weight bf16 matmuls on TensorE. The causal depthwise conv is
folded into the u-projection as 4 tap-scaled shifted matmuls. All projection
matmuls are bf16 with f32 PSUM accumulation.
"""
import sys

if '/opt/trn_rl_repo' not in sys.path:
    sys.path.insert(0, '/opt/trn_rl_repo')

import ml_dtypes
import numpy as np

import concourse.bass as bass
import concourse.bacc as bacc
import concourse.tile as tile
from concourse import mybir
from concourse.bass_utils import run_bass_kernel_spmd

HID = 128
BATCH = 4
SEQ = 2048
D_STATE = 16
D_CONV = 4
D_INNER = 256
DT_RANK = 8

L = SEQ
C = HID
CW = 512           # matmul / PSUM chunk width
NCH = L // CW
NS = 16            # states; one fused [128, 2*L] scan per state
f32 = mybir.dt.float32
bf16 = mybir.dt.bfloat16
mult = mybir.AluOpType.mult
add = mybir.AluOpType.add
sub = mybir.AluOpType.subtract
AF = mybir.ActivationFunctionType

_PROGRAM = None


def _declare(nc):
    def dp(name, shape, dt=f32):
        return nc.declare_dram_parameter(name, list(shape), dt, isOutput=False)
    p = {}
    p['x'] = dp('x', (C, L))
    for n in ('wlgT', 'wcmT', 'loT'):
        p[n] = dp(n, (C, C), bf16)
    # conv folded into the u-projection: 4 tap-scaled copies of in_w's u-half
    p['inwuT'] = dp('inwuT', (C, D_CONV * 2 * 128), bf16)
    p['inwzT'] = dp('inwzT', (C, 2 * 128), bf16)
    p['xpwT0'] = dp('xpwT0', (128, 80), bf16)   # dtr @0:8, B @32:48, C @64:80
    p['xpwT1'] = dp('xpwT1', (128, 80), bf16)
    p['dtwT'] = dp('dtwT', (DT_RANK, D_INNER), bf16)
    p['outwT0'] = dp('outwT0', (128, C), bf16)
    p['outwT1'] = dp('outwT1', (128, C), bf16)
    p['acols'] = dp('acols', (128, 2 * NS))     # col 2s+h = A[128h:128(h+1), s]
    p['iden'] = dp('iden', (128, 128), bf16)
    for n in ('conv_b', 'dt_b', 'dp_v'):
        p[n] = dp(n, (128, 2))                  # halves in columns
    for n in ('bias_lg', 'bias_cm', 'lo_b', 'ln_g', 'ln_b', 'm_fwd', 'm_bwd'):
        p[n] = dp(n, (C, 1))
    p['y'] = nc.declare_dram_parameter('y', [C, L], f32, isOutput=True)
    return p


class B:
    """Builder state shared by the stage helpers."""


def _proj(b, ps_pool, lhsT, rhs, out, func, bias, out_cols=None, rows=C):
    """out[:, cs] = func(lhsT.T @ rhs[:, cs] + bias) per CW-chunk (PE + ACT)."""
    nc = b.nc
    for ci in range(NCH):
        cs = slice(ci * CW, (ci + 1) * CW)
        ocs = cs if out_cols is None else slice(out_cols + ci * CW, out_cols + (ci + 1) * CW)
        ps = ps_pool.tile([rows, CW], f32, name='bank', tag='bank')
        nc.tensor.matmul(ps, lhsT, rhs[:, cs], start=True, stop=True)
        nc.scalar.activation(out[:, ocs], ps, func, bias=bias)


def _layernorm(b, ps_pool, pool, x_sb, out, pref, width=L):
    """LayerNorm over the 128 channels per column into `out` (any dtype):
    (x - mean) * rsqrt(var + eps). Stats via bf16 ones-matmuls; the mean/rstd
    rows are broadcast back across partitions with K=1 ones-row matmuls.
    Stage-major emission so the in-order engines pipeline across chunks and
    the Ln/Exp activation-table loads happen once, not per chunk."""
    nc = b.nc
    nch = width // CW
    xb = pool.tile([C, width], bf16, name=f'lnxb{pref}', tag=f'lnxb{pref}')
    ex = pool.tile([1, width], bf16, name=f'lnex{pref}', tag=f'lnex{pref}')
    rr_ = pool.tile([1, width], f32, name=f'lnrr{pref}', tag=f'lnrr{pref}')
    nrm0 = pool.tile([C, width], f32, name=f'nrm0{pref}', tag=f'nrm0{pref}')
    sq2 = pool.tile([C, width], bf16, name=f'sq2{pref}', tag=f'sq2{pref}')
    cslices = [slice(ci * CW, (ci + 1) * CW) for ci in range(nch)]
    for cs in cslices:
        nc.vector.tensor_copy(xb[:, cs], x_sb[:, cs])
    ps0s = [ps_pool.tile([1, CW], f32, name='bank', tag='bank') for _ in cslices]
    for cs, ps0 in zip(cslices, ps0s):
        nc.tensor.matmul(ps0, b.ones_col, xb[:, cs], start=True, stop=True)
    for cs, ps0 in zip(cslices, ps0s):
        nc.scalar.activation(ex[:, cs], ps0, AF.Identity, bias=0.0, scale=1.0 / C)
    for ci, cs in enumerate(cslices):
        psb = ps_pool.tile([128, CW], f32, name='bank', tag='bank')
        nc.tensor.matmul(psb, b.ones_row, ex[:, cs], start=True, stop=True)
        nc.vector.scalar_tensor_tensor(nrm0[:, cs], x_sb[:, cs], 1.0, psb, mult, sub)
        nc.vector.tensor_tensor(sq2[:, cs], nrm0[:, cs], nrm0[:, cs], mult)
    psvs = [ps_pool.tile([1, CW], f32, name='bank', tag='bank') for _ in cslices]
    for cs, psv in zip(cslices, psvs):
        nc.tensor.matmul(psv, b.ones_col, sq2[:, cs], start=True, stop=True)
    for cs, psv in zip(cslices, psvs):
        nc.scalar.activation(rr_[:, cs], psv, AF.Ln, bias=b.eps_t[:, :], scale=1.0 / C)
    for cs in cslices:
        nc.scalar.activation(rr_[:, cs], rr_[:, cs], AF.Exp, bias=0.0, scale=-0.5)
    for cs in cslices:
        psr = ps_pool.tile([128, CW], f32, name='bank', tag='bank')
        nc.tensor.matmul(psr, b.ones_row_f, rr_[:, cs], start=True, stop=True)
        nc.vector.scalar_tensor_tensor(out[:, cs], nrm0[:, cs], 1.0, psr, mult, mult)


def _build_body(nc, tc, p, ctx):
    b = B()
    b.nc = nc
    io = ctx.enter_context(tc.tile_pool(name='io', bufs=1))
    b.dram = ctx.enter_context(tc.tile_pool(name='drm', bufs=1, space='DRAM'))

    # x first: its DMA leads the dispatch queue so LN1 starts immediately
    x = io.tile([C, L], f32, name='x', tag='x')
    nc.sync.dma_start(out=x, in_=p['x'][:, :])

    # ---- load weights/vectors (persistent) ----
    W = {}
    for n, shape, dt in (('wlgT', (C, C), bf16),
                         ('wcmT', (C, C), bf16), ('loT', (C, C), bf16),
                         ('inwuT', (C, D_CONV * 2 * 128), bf16),
                         ('inwzT', (C, 2 * 128), bf16),
                         ('xpwT0', (128, 80), bf16), ('xpwT1', (128, 80), bf16),
                         ('dtwT', (8, 256), bf16),
                         ('outwT0', (128, C), bf16), ('outwT1', (128, C), bf16),
                         ('acols', (128, 2 * NS), f32), ('iden', (128, 128), bf16)):
        W[n] = io.tile(list(shape), dt, name=n, tag=n)
        nc.sync.dma_start(out=W[n], in_=p[n][:, :])
    V = {}
    for n in ('conv_b', 'dt_b', 'dp_v'):
        V[n] = io.tile([128, 2], f32, name=n, tag=n)
        nc.sync.dma_start(out=V[n], in_=p[n][:, :])
    for n in ('bias_lg', 'bias_cm', 'lo_b', 'ln_g', 'ln_b', 'm_fwd', 'm_bwd'):
        V[n] = io.tile([C, 1], f32, name=n, tag=n)
        nc.sync.dma_start(out=V[n], in_=p[n][:, :])
    ones_col = io.tile([C, 1], bf16, name='ones_col', tag='ones_col')
    nc.vector.memset(ones_col, 1.0)
    b.ones_col = ones_col
    eps_t = io.tile([1, 1], f32, name='lneps', tag='lneps')
    nc.vector.memset(eps_t, 1e-5)
    b.eps_t = eps_t
    ones_row = io.tile([1, 128], bf16, name='ones_row', tag='ones_row')
    nc.vector.memset(ones_row, 1.0)
    b.ones_row = ones_row
    ones_row_f = io.tile([1, 128], f32, name='ones_row_f', tag='ones_row_f')
    nc.vector.memset(ones_row_f, 1.0)
    b.ones_row_f = ones_row_f

    # persistent activations that survive into the s-loop / tail
    gate = io.tile([C, L], bf16, name='gate', tag='gate')
    b.nrm = io.tile([C, L], bf16, name='nrm', tag='nrm')
    uc = [io.tile([128, L], bf16, name=f'uc{h}', tag=f'uc{h}') for h in range(2)]
    sz_t = [io.tile([128, L], bf16, name=f'sz{h}', tag=f'sz{h}') for h in range(2)]
    dtt = [io.tile([128, L], bf16, name=f'dtt{h}', tag=f'dtt{h}') for h in range(2)]
    dtut = [io.tile([128, L], bf16, name=f'dtut{h}', tag=f'dtut{h}') for h in range(2)]

    b_d = b.dram.tile([NS, L], bf16, name='b_d', tag='b_d')
    c_d = b.dram.tile([NS, L], bf16, name='c_d', tag='c_d')

    # ================= P1/P2: layernorm, projections, conv, dbl =============
    with tc.tile_pool(name='head', bufs=1) as head, \
         tc.tile_pool(name='hps', bufs=4, space='PSUM') as hps:
        _layernorm(b, hps, head, x, b.nrm, 'l1')

        # lm-projection folded into wc on the host (both are linear):
        # xm = silu((wc @ wlm') @ nrm + (wc @ b_lm + cb)),
        # left-padded with D_CONV-1 zero columns for the folded conv
        xmp = head.tile([C, D_CONV - 1 + L], bf16, name='xmp', tag='xmp')
        nc.vector.memset(xmp[:, 0:D_CONV - 1], 0.0)
        _proj(b, hps, W['wcmT'], b.nrm, xmp, AF.Silu, V['bias_cm'][:, :],
              out_cols=D_CONV - 1)

        # z-projection + silu, and the u-projection with the causal depthwise
        # conv folded in: uc[:, t] = silu(sum_k (cw_k*in_w_u) @ xm[:, t-3+k] + cb)
        for h in range(2):
            _proj(b, hps, W['inwzT'][:, 128 * h:128 * (h + 1)], xmp[:, 3:3 + L],
                  sz_t[h], AF.Silu, 0.0)
            for ci in range(NCH):
                cs = slice(ci * CW, (ci + 1) * CW)
                ps_u = hps.tile([128, CW], f32, name='bank', tag='bank')
                for kk in range(D_CONV):
                    wk = W['inwuT'][:, 128 * (4 * h + kk):128 * (4 * h + kk + 1)]
                    nc.tensor.matmul(ps_u, wk, xmp[:, ci * CW + kk:ci * CW + kk + CW],
                                     start=(kk == 0), stop=(kk == D_CONV - 1))
                nc.scalar.activation(uc[h][:, cs], ps_u, AF.Silu,
                                     bias=V['conv_b'][:, h:h + 1])

        # dbl = xp_w @ uc -> dtr(8, bf16), B(16, bf16), Cm(16, bf16)
        dtr = head.tile([8, L], bf16, name='dtr', tag='dtr')
        b_sb = head.tile([16, L], bf16, name='b_sb', tag='b_sb')
        c_sb = head.tile([16, L], bf16, name='c_sb', tag='c_sb')
        for ci in range(NCH):
            cs = slice(ci * CW, (ci + 1) * CW)
            ps_dbl = hps.tile([80, CW], f32, name='bank', tag='bank')
            nc.tensor.matmul(ps_dbl, W['xpwT0'], uc[0][:, cs], start=True, stop=False)
            nc.tensor.matmul(ps_dbl, W['xpwT1'], uc[1][:, cs], start=False, stop=True)
            nc.vector.tensor_copy(dtr[:, cs], ps_dbl[0:8, :])
            nc.vector.tensor_copy(b_sb[:, cs], ps_dbl[32:48, :])
            nc.vector.tensor_copy(c_sb[:, cs], ps_dbl[64:80, :])
            # stash B/C chunks to DRAM for the per-state partition broadcast
            nc.sync.dma_start(out=b_d[:, cs], in_=b_sb[:, cs])
            nc.sync.dma_start(out=c_d[:, cs], in_=c_sb[:, cs])

        # dt = softplus(dt_w @ dtr + dt_b) (bf16); dtu = dt * uc
        # softplus(z) = ln(1 + exp(z)) -- no softplus entry in the ACT tables.
        # Stage-major so the Exp/Ln table loads happen once each.
        for h in range(2):
            for ci in range(NCH):
                cs = slice(ci * CW, (ci + 1) * CW)
                ps_dt = hps.tile([128, CW], f32, name='bank', tag='bank')
                nc.tensor.matmul(ps_dt, W['dtwT'][:, 128 * h:128 * (h + 1)],
                                 dtr[:, cs], start=True, stop=True)
                nc.scalar.activation(dtt[h][:, cs], ps_dt, AF.Exp,
                                     bias=V['dt_b'][:, h:h + 1])
        for h in range(2):
            nc.scalar.activation(dtt[h], dtt[h], AF.Ln, bias=1.0, scale=1.0)
            nc.vector.tensor_tensor(dtut[h], dtt[h], uc[h], mult)

    # ================= P3: selective scan, one fused tile per state =========
    yz = []
    with tc.tile_pool(name='py', bufs=1, space='PSUM') as py, \
         tc.tile_pool(name='rot', bufs=3) as rot:
        psy = [py.tile([128, L], f32, name=f'psy{h}', tag=f'psy{h}') for h in range(2)]
        for s in range(NS):
            b_bc = rot.tile([128, L], bf16, name='b_bc', tag='b_bc')
            src = bass.AP(tensor=b_d.tensor, offset=b_d.offset + s * L,
                          ap=[[0, 128], [1, L]])
            nc.sync.dma_start(out=b_bc, in_=src)
            c_bc = rot.tile([128, L], bf16, name='c_bc', tag='c_bc')
            src = bass.AP(tensor=c_d.tensor, offset=c_d.offset + s * L,
                          ap=[[0, 128], [1, L]])
            nc.gpsimd.dma_start(out=c_bc, in_=src)

            da = rot.tile([128, 2 * L], f32, name='da', tag='da')
            nc.scalar.activation(da[:, 0:L], dtt[0], AF.Exp, bias=0.0,
                                 scale=W['acols'][:, 2 * s:2 * s + 1])
            nc.scalar.activation(da[:, L + 1:2 * L], dtt[1][:, 1:L], AF.Exp,
                                 bias=0.0, scale=W['acols'][:, 2 * s + 1:2 * s + 2])
            # state reset at the half boundary: h_first = 0*h_prev + dBu_first
            nc.vector.memset(da[:, L:L + 1], 0.0)

            dbu = rot.tile([128, 2 * L], bf16, name='dbu', tag='dbu')
            nc.vector.tensor_tensor(dbu[:, 0:L], dtut[0], b_bc, mult)
            nc.vector.tensor_tensor(dbu[:, L:2 * L], dtut[1], b_bc, mult)

            ht = rot.tile([128, 2 * L], bf16, name='ht', tag='ht')
            nc.vector.tensor_tensor_scan(ht, da, dbu, 0.0, mult, add)

            ycm = rot.tile([128, 2 * L], bf16, name='ycm', tag='ycm')
            nc.vector.tensor_tensor(ycm[:, 0:L], ht[:, 0:L], c_bc, mult)
            nc.vector.tensor_tensor(ycm[:, L:2 * L], ht[:, L:2 * L], c_bc, mult)

            for h in range(2):
                for ci in range(NCH):
                    ics = slice(h * L + ci * CW, h * L + (ci + 1) * CW)
                    ocs = slice(ci * CW, (ci + 1) * CW)
                    nc.tensor.matmul(psy[h][:, ocs], W['iden'], ycm[:, ics],
                                     start=(s == 0), stop=(s == NS - 1),
                                     skip_group_check=True)

        # y1 = uc*Dp + psy ; yz = y1 * silu(z)
        for h in range(2):
            yzt = io.tile([128, L], bf16, name=f'yz{h}', tag=f'yz{h}')
            nc.vector.scalar_tensor_tensor(
                yzt, uc[h], V['dp_v'][:, h:h + 1], psy[h], mult, add)
            nc.vector.tensor_tensor(yzt, yzt, sz_t[h], mult)
            yz.append(yzt)

    # ================= P4: out-proj, flip, select, pairwise exchange ========
    with tc.tile_pool(name='tail', bufs=1) as tail, \
         tc.tile_pool(name='tps', bufs=4, space='PSUM') as tps:
        y_dir = tail.tile([C, L], bf16, name='y_dir', tag='y_dir')
        for ci in range(NCH):
            cs = slice(ci * CW, (ci + 1) * CW)
            ps_o = tps.tile([C, CW], f32, name='bank', tag='bank')
            nc.tensor.matmul(ps_o, W['outwT0'], yz[0][:, cs], start=True, stop=False)
            nc.tensor.matmul(ps_o, W['outwT1'], yz[1][:, cs], start=False, stop=True)
            nc.scalar.activation(y_dir[:, cs], ps_o, AF.Identity, bias=0.0)

        y_flip = tail.tile([C, L], bf16, name='y_flip', tag='y_flip')
        nc.vector.tensor_copy(y_flip, y_dir[:, ::-1])
        y_sel = tail.tile([C, L], bf16, name='y_sel', tag='y_sel')
        nc.vector.tensor_scalar_mul(y_sel, y_dir, V['m_fwd'][:, :])
        nc.vector.scalar_tensor_tensor(y_sel, y_flip, V['m_bwd'][:, :], y_sel, mult, add)

        # pairwise ReduceScatter over column halves: even cores get summed
        # cols 0:L/2, odd cores cols L/2:L; the host stitches the halves.
        LH = L // 2
        cc_in = b.dram.tile([2 * C, LH], bf16, name='cc_in', tag='cc_in')
        cc_out = b.dram.tile([C, LH], bf16, name='cc_out', tag='cc_out')
        nc.sync.dma_start(out=cc_in[0:C, :], in_=y_sel[:, 0:LH])
        nc.sync.dma_start(out=cc_in[C:2 * C, :], in_=y_sel[:, LH:L])
        nc.gpsimd.collective_compute(
            'ReduceScatter', add,
            replica_groups=[[0, 1], [2, 3], [4, 5], [6, 7]],
            ins=[cc_in.opt()], outs=[cc_out.opt()])
        # gate projection scheduled here so PE/ACT run it in the CC's shadow
        _proj(b, tps, W['wlgT'], b.nrm, gate, AF.Silu, V['bias_lg'][:, :])
        # core-parity column half of the gate, via the fwd/bwd masks
        ghalf = tail.tile([C, LH], bf16, name='ghalf', tag='ghalf')
        nc.vector.tensor_scalar_mul(ghalf, gate[:, 0:LH], V['m_fwd'][:, :])
        nc.vector.scalar_tensor_tensor(ghalf, gate[:, LH:L], V['m_bwd'][:, :],
                                       ghalf, mult, add)
        y_sum = tail.tile([C, LH], bf16, name='y_sum', tag='y_sum')
        nc.sync.dma_start(out=y_sum, in_=cc_out)

        # ---- P5 tail (half-width): gate multiply, lo projection, final LN --
        g1 = tail.tile([C, LH], bf16, name='g1', tag='g1')
        nc.vector.tensor_tensor(g1, y_sum, ghalf, mult)
        t2 = tail.tile([C, LH], f32, name='t2', tag='t2')
        for ci in range(LH // CW):
            cs = slice(ci * CW, (ci + 1) * CW)
            ps_l = tps.tile([C, CW], f32, name='bank', tag='bank')
            nc.tensor.matmul(ps_l, W['loT'], g1[:, cs], start=True, stop=True)
            nc.scalar.activation(t2[:, cs], ps_l, AF.Identity, bias=V['lo_b'][:, :])

        o1 = tail.tile([C, LH], f32, name='o1', tag='o1')
        _layernorm(b, tps, tail, t2, o1, 'l2', width=LH)
        out_sb = tail.tile([C, LH], f32, name='out_sb', tag='out_sb')
        nc.scalar.activation(out_sb, o1, AF.Identity, bias=V['ln_b'][:, :],
                             scale=V['ln_g'][:, :])
        nc.sync.dma_start(out=p['y'][:, 0:LH], in_=out_sb)


def _build_program():
    import contextlib
    nc = bacc.Bacc('TRN2', target_bir_lowering=False, debug=False, num_devices=8)
    p = _declare(nc)
    with tile.TileContext(nc) as tc:
        with contextlib.ExitStack() as ctx:
            _build_body(nc, tc, p, ctx)
    nc.compile()
    return nc


def _prep_core_inputs(inputs, bidx, d):
    g = lambda n: np.asarray(inputs[n], dtype=np.float32)
    x = g('x')
    ln_g = g('ln_g')
    ln_b = g('ln_b')
    pre = 'mf_' if d == 0 else 'mb_'
    P = lambda n: np.asarray(inputs[pre + n], dtype=np.float32)

    lm_w, lm_b = g('lm_w'), g('lm_b')
    lg_w, lg_b = g('lg_w'), g('lg_b')
    lo_w, lo_b = g('lo_w'), g('lo_b')
    if d == 0:
        wc, cb = g('cf_w'), g('cf_b')
    else:
        wc, cb = np.ascontiguousarray(g('cb_w')[:, ::-1]), g('cb_b')

    A = -np.exp(P('Alog'))                       # (256,16)
    acols = np.zeros((128, 2 * NS), np.float32)  # col 2s+h = A[128h:128(h+1), s]
    for s in range(NS):
        acols[:, 2 * s] = A[0:128, s]
        acols[:, 2 * s + 1] = A[128:256, s]

    halves = lambda v: np.ascontiguousarray(
        np.stack([v[:128], v[128:]], axis=1).astype(np.float32))
    col = lambda v: np.ascontiguousarray(v.astype(np.float32).reshape(-1, 1))
    bf = lambda w: np.ascontiguousarray(w).astype(ml_dtypes.bfloat16)
    T = lambda w: np.ascontiguousarray(w.T.astype(np.float32))

    xpwT40 = np.ascontiguousarray(P('xp_w').T)   # (256,40)
    xpwT = np.zeros((256, 80), np.float32)       # 32-aligned sections for PSUM reads
    xpwT[:, 0:8] = xpwT40[:, 0:8]
    xpwT[:, 32:48] = xpwT40[:, 8:24]
    xpwT[:, 64:80] = xpwT40[:, 24:40]
    outwT = np.ascontiguousarray(P('out_w').T)   # (256,128)
    cwn = P('conv_w')                            # (256,4)
    inw = P('in_w')                              # (512,128): u rows 0:256, z 256:512
    # conv folded into the u-projection: per half h and tap k, (cw_k * in_w_u).T
    inwuT = np.concatenate(
        [np.ascontiguousarray((cwn[128 * h:128 * (h + 1), kk:kk + 1]
                               * inw[128 * h:128 * (h + 1)]).T)
         for h in range(2) for kk in range(D_CONV)], axis=1)
    inwzT = np.ascontiguousarray(inw[256:512].T)

    return {
        'x': np.ascontiguousarray(x[bidx]),
        'wlgT': bf(T(lg_w * ln_g[None, :])),
        'wcmT': bf(T(wc @ (lm_w * ln_g[None, :]))),
        'loT': bf(T(lo_w)),
        'inwuT': bf(inwuT),
        'inwzT': bf(inwzT),
        'xpwT0': bf(xpwT[:128]),
        'xpwT1': bf(xpwT[128:]),
        'dtwT': bf(np.ascontiguousarray(P('dt_w').T)),
        'outwT0': bf(outwT[:128]),
        'outwT1': bf(outwT[128:]),
        'acols': acols,
        'iden': bf(np.eye(128, dtype=np.float32)),
        'conv_b': halves(P('conv_b')),
        'dt_b': halves(P('dt_b')),
        'dp_v': halves(P('D')),
        'bias_lg': col(lg_w @ ln_b + lg_b),
        'bias_cm': col(wc @ (lm_w @ ln_b + lm_b) + cb),
        'lo_b': col(lo_b),
        'ln_g': col(ln_g),
        'ln_b': col(ln_b),
        'm_fwd': np.full((C, 1), 1.0 if d == 0 else 0.0, np.float32),
        'm_bwd': np.full((C, 1), 0.0 if d == 0 else 1.0, np.float32),
    }


def get_program():
    global _PROGRAM
    if _PROGRAM is None:
        _PROGRAM = _build_program()
    return _PROGRAM


def run(inputs, **run_kwargs):
    nc = get_program()
    in_maps = [_prep_core_inputs(inputs, c // 2, c % 2) for c in range(8)]
    res = run_bass_kernel_spmd(nc, in_maps, core_ids=list(range(8)), **run_kwargs)
    # each pair's even core computes output cols 0:L/2, the odd core L/2:L
    out = np.stack(
        [np.concatenate([res.results[2 * b]['y'][:, :L // 2],
                         res.results[2 * b + 1]['y'][:, :L // 2]], axis=1)
         for b in range(BATCH)], axis=0)
    return out, res


def kernel(**inputs) -> np.ndarray:
    out, _ = run(inputs)
    return out.astype(np.float32)
